# revision 1
# baseline (speedup 1.0000x reference)
"""MoE (15 routed experts top-3 + shared GEGLU FFN) on 8 trn2 NeuronCores.

Strategy (expert-parallel + shared-expert tensor-parallel):
  - Each core owns 2 routed experts (core 7: 1 real + 1 zero dummy) and a
    256-wide slice of the shared expert's FS=2048 hidden dim.
  - Gate is computed replicated on every core in compensated bf16 (4-term
    split-product, ~1e-7 error); per-core input permutation puts the core's
    own experts in gate columns 0/1.
  - Token dispatch is built on-device with matmuls: top-3 via max8,
    per-expert cumsum via a triangular matmul, then a selection-matrix
    matmul extracts (token-id, weight) per capacity slot.
  - Experts run on gathered tokens only (capacity 512/expert) in bf16;
    combine is an indirect scatter-add DMA into the output.
  - Host sums the 8 partial outputs.
"""

import sys
import numpy as np

for _p in ("/opt/trn_rl_repo",):
    if _p not in sys.path:
        sys.path.insert(0, _p)

import ml_dtypes

S, B, D = 1024, 2, 1024
T = S * B                  # 2048 tokens
E, TOPK = 15, 3
F, FS = 1024, 2048
NC = 8                     # cores
EPC = 2                    # expert slots per core
CAP = 512                  # per-expert token capacity (max actual count ~463)
FSS = FS // NC             # shared-expert hidden slice per core = 256
NEG = -1.0e9

P = 128
DKT = D // P               # 8 k-tiles over D
FKT = F // P               # 8 k-tiles over F
NT = T // P                # 16 token tiles
NMT = CAP // P             # 4 capacity (slot) tiles per expert
NFT = 2 * F // P           # 16 f-tiles of fc1 output

_prog_cache = {}


# ----------------------------------------------------------------------------
# device program
# ----------------------------------------------------------------------------

def build_program():
    import concourse.bass as bass
    import concourse.mybir as mybir
    import concourse.tile as tile
    from concourse import bacc
    from concourse.masks import make_identity

    fp32 = mybir.dt.float32
    bf16 = mybir.dt.bfloat16
    i32 = mybir.dt.int32

    nc = bacc.Bacc()

    xbf = nc.dram_tensor("xbf", [T, D], bf16, kind="ExternalInput")
    xer = nc.dram_tensor("xer", [T, D], bf16, kind="ExternalInput")
    gwb = nc.dram_tensor("gwb", [D, 16], bf16, kind="ExternalInput")
    gwe = nc.dram_tensor("gwe", [D, 16], bf16, kind="ExternalInput")
    gbias = nc.dram_tensor("gbias", [P, 16], fp32, kind="ExternalInput")
    ltm = nc.dram_tensor("ltm", [P, P], fp32, kind="ExternalInput")
    w1t = nc.dram_tensor("w1t", [EPC, NFT, P, DKT, P], bf16, kind="ExternalInput")
    b1 = nc.dram_tensor("b1", [P, EPC, NFT], fp32, kind="ExternalInput")
    w2t = nc.dram_tensor("w2t", [EPC, P, FKT, D], bf16, kind="ExternalInput")
    b2 = nc.dram_tensor("b2", [1, EPC, D], fp32, kind="ExternalInput")
    s1wt = nc.dram_tensor("s1wt", [P, DKT, 2 * FSS], bf16, kind="ExternalInput")
    s1b = nc.dram_tensor("s1b", [P, 4], fp32, kind="ExternalInput")
    s2wt = nc.dram_tensor("s2wt", [P, FSS // P, D], bf16, kind="ExternalInput")
    s2b = nc.dram_tensor("s2b", [1, D], fp32, kind="ExternalInput")
    out = nc.dram_tensor("out", [T, D], fp32, kind="ExternalOutput")

    with tile.TileContext(nc) as tc:
        emit(nc, tc, tile, mybir, bass, make_identity, fp32, bf16, i32,
             dict(xbf=xbf, xer=xer, gwb=gwb, gwe=gwe, gbias=gbias, ltm=ltm,
                  w1t=w1t, b1=b1, w2t=w2t, b2=b2, s1wt=s1wt, s1b=s1b,
                  s2wt=s2wt, s2b=s2b, out=out))
    if not nc.is_finalized():
        nc.finalize()
    return nc


def emit(nc, tc, tile, mybir, bass, make_identity, fp32, bf16, i32, io):
    from contextlib import ExitStack

    AF = mybir.ActivationFunctionType
    OP = mybir.AluOpType
    xbf, out = io["xbf"], io["out"]

    ctx = ExitStack()
    with ctx:
        consts = ctx.enter_context(tc.tile_pool(name="consts", bufs=1))
        wpool = ctx.enter_context(tc.tile_pool(name="weights", bufs=1))
        xbt_pool = ctx.enter_context(tc.tile_pool(name="xbt", bufs=1))
        w1pool = ctx.enter_context(tc.tile_pool(name="w1", bufs=4))
        sb = ctx.enter_context(tc.tile_pool(name="sb", bufs=2))
        ysp = ctx.enter_context(tc.tile_pool(name="ysp", bufs=2))
        xgp = ctx.enter_context(tc.tile_pool(name="xgp", bufs=3))
        small = ctx.enter_context(tc.tile_pool(name="small", bufs=4))
        persist = ctx.enter_context(tc.tile_pool(name="persist", bufs=1))
        apool = ctx.enter_context(tc.tile_pool(name="apool", bufs=2))
        ycpool = ctx.enter_context(tc.tile_pool(name="ycpool", bufs=2))

        # ---- constants / weights staged to SBUF ----
        ident = consts.tile([P, P], fp32)
        make_identity(nc, ident[:])
        ident_bf = consts.tile([P, P], bf16)
        make_identity(nc, ident_bf[:])
        ones_col = consts.tile([1, P], fp32)
        nc.vector.memset(ones_col[:], 1.0)
        ones_colp = consts.tile([P, 1], fp32)
        nc.vector.memset(ones_colp[:], 1.0)

        # PE warm-up: ~100 dummy transposes during the DMA-bound startup keep
        # the HAM activity monitor busy so real matmuls start at 2.4GHz.
        with tc.tile_pool(name="warm", bufs=2, space="PSUM") as warm:
            for _ in range(100):
                wt = warm.tile([P, P], bf16, tag="wt")
                nc.tensor.transpose(wt[:], ident_bf[:], ident_bf[:])

        # persistent activations
        xbt = xbt_pool.tile([P, 4, DKT, 512], bf16)  # x^T in token quarters
        comb = persist.tile([P, NT, 16], fp32)      # renormalized top-3 weights

        gw2_sb = consts.tile([P, DKT, 48], bf16)   # gwb at M 0-15, gwe at M 32-47
        nc.vector.memset(gw2_sb[:], 0)
        nc.sync.dma_start(out=gw2_sb[:, :, 0:16], in_=io["gwb"].rearrange("(kt p) e -> p kt e", p=P))
        nc.sync.dma_start(out=gw2_sb[:, :, 32:48], in_=io["gwe"].rearrange("(kt p) e -> p kt e", p=P))
        gbias_sb = consts.tile([P, 16], fp32)
        nc.sync.dma_start(out=gbias_sb[:], in_=io["gbias"][:])
        lt_sb = consts.tile([P, P], fp32)
        nc.sync.dma_start(out=lt_sb[:], in_=io["ltm"][:])
        b1_sb = consts.tile([P, EPC, NFT], fp32)
        nc.sync.dma_start(out=b1_sb[:], in_=io["b1"][:])
        b2_sb = consts.tile([1, EPC, D], fp32)
        nc.sync.dma_start(out=b2_sb[:], in_=io["b2"][:])
        s1b_sb = consts.tile([P, 4], fp32)
        nc.sync.dma_start(out=s1b_sb[:], in_=io["s1b"][:])
        s2b_sb = consts.tile([1, D], fp32)
        nc.sync.dma_start(out=s2b_sb[:], in_=io["s2b"][:])


        # sync-queue order: transposes interleaved with weight loads so both
        # the gate (xbt/xet) and shared fc1 (s1w) can start by ~15-20us.
        xet_pool = ctx.enter_context(tc.tile_pool(name="xet_pool", bufs=2))
        xet_t = []
        s1w_sb = wpool.tile([P, DKT, 2 * FSS], bf16)
        s2w_sb = wpool.tile([P, FSS // P, D], bf16)
        w2_sb = [wpool.tile([P, FKT, D], bf16, tag=f"w2_{le}", name=f"w2_{le}")
                 for le in range(EPC)]

        def _trans(q):
            nc.sync.dma_start_transpose(xbt[:, q], xbf[q * 512:(q + 1) * 512, :])
            xet = xet_pool.tile([P, DKT, 512], bf16, tag="xet", name=f"xet{q}")
            nc.sync.dma_start_transpose(xet[:], io["xer"][q * 512:(q + 1) * 512, :])
            xet_t.append(xet)

        _trans(0)
        nc.sync.dma_start(out=s1w_sb[:], in_=io["s1wt"][:])
        _trans(1)
        nc.sync.dma_start(out=s2w_sb[:], in_=io["s2wt"][:])
        _trans(2)
        _trans(3)
        for le in range(EPC):
            nc.sync.dma_start(out=w2_sb[le][:], in_=io["w2t"][le])

        iota512f = consts.tile([P, CAP], fp32)
        with tc.tile_pool(name="iota_tmp", bufs=1) as iota_tmp:
            iota512 = iota_tmp.tile([P, CAP], i32)
            nc.gpsimd.iota(iota512[:], pattern=[[1, CAP]], base=0, channel_multiplier=0)
            nc.vector.tensor_copy(iota512f[:], iota512[:])
        ktid = consts.tile([P, NT], i32)
        nc.gpsimd.iota(ktid[:], pattern=[[1, NT]], base=0, channel_multiplier=0)
        ktid_bf = consts.tile([P, NT], bf16)
        nc.vector.tensor_copy(ktid_bf[:], ktid[:])
        pid = consts.tile([P, NT], i32)
        nc.gpsimd.iota(pid[:], pattern=[[0, NT]], base=0, channel_multiplier=1)
        pid_bf = consts.tile([P, NT], bf16)
        nc.vector.tensor_copy(pid_bf[:], pid[:])

        # broadcast bias rows (ones outer-product), computed once on PE
        bbc = ctx.enter_context(tc.tile_pool(name="bbc", bufs=1))
        s2bc = bbc.tile([P, D], bf16)
        b2bc = bbc.tile([P, EPC, D], bf16)
        with tc.tile_pool(name="pbc", bufs=2, space="PSUM") as pbc:
            for h in range(2):
                hs = slice(h * 512, (h + 1) * 512)
                pc = pbc.tile([P, 512], fp32, tag="pc")
                nc.tensor.matmul(pc[:], lhsT=ones_col[:], rhs=s2b_sb[:, hs],
                                 start=True, stop=True)
                nc.vector.tensor_copy(s2bc[:, hs], pc[:])
                for le in range(EPC):
                    pc2 = pbc.tile([P, 512], fp32, tag="pc")
                    nc.tensor.matmul(pc2[:], lhsT=ones_col[:], rhs=b2_sb[:, le, hs],
                                     start=True, stop=True)
                    nc.vector.tensor_copy(b2bc[:, le, hs], pc2[:])


        # ------------------------------------------------------------------
        # Phase 1: compensated-bf16 gate -> logits tiles -> top-3 weights
        # ------------------------------------------------------------------
        with tc.tile_pool(name="p1lt", bufs=3, space="PSUM") as p1lt, \
             tc.tile_pool(name="p1tr", bufs=2, space="PSUM") as p1tr:
            for ch in range(T // 512):
                cs = slice(ch * 512, (ch + 1) * 512)
                xet = xet_t[ch]
                plt2 = p1lt.tile([48, 512], fp32, tag="plt2")
                for kt in range(DKT):
                    nc.tensor.matmul(plt2[:], lhsT=gw2_sb[:, kt, :],
                                     rhs=xbt[:, ch, kt, :],
                                     start=(kt == 0), stop=(kt == DKT - 1))
                for kt in range(DKT):
                    nc.tensor.matmul(plt2[:16, :], lhsT=gw2_sb[:, kt, 0:16],
                                     rhs=xet[:, kt, :],
                                     start=False, stop=(kt == DKT - 1),
                                     skip_group_check=True)
                lgt_e = sb.tile([16, 512], fp32, tag="lgt", name="lgt_e")
                nc.scalar.copy(lgt_e[:], plt2[32:48, :])
                lgt = sb.tile([16, 512], fp32, tag="lgt")
                nc.vector.tensor_add(lgt[:], plt2[:16, :], lgt_e[:])
                for q in range(4):
                    ci = ch * 4 + q
                    ptr = p1tr.tile([P, 16], fp32, tag="ptr")
                    nc.tensor.transpose(ptr[:], lgt[:, q * P:(q + 1) * P],
                                        ident[:16, :16])
                    lg = sb.tile([P, 16], fp32, tag="lg")
                    nc.vector.tensor_add(lg[:], ptr[:], gbias_sb[:])
                    mx8 = small.tile([P, 8], fp32, tag="mx8")
                    nc.vector.max(out=mx8[:], in_=lg[:])
                    dd = small.tile([P, 16], fp32, tag="dd")
                    nc.vector.tensor_scalar(dd[:], lg[:], mx8[:, 0:1], None,
                                            op0=OP.subtract)
                    ee = small.tile([P, 16], fp32, tag="ee")
                    nc.scalar.activation(ee[:], dd[:], AF.Exp)
                    mm = small.tile([P, 16], fp32, tag="mm")
                    nc.vector.tensor_scalar(mm[:], lg[:], mx8[:, 2:3], None,
                                            op0=OP.is_ge)
                    we = small.tile([P, 16], fp32, tag="we")
                    nc.vector.tensor_mul(we[:], ee[:], mm[:])
                    ss = small.tile([P, 1], fp32, tag="ss")
                    nc.vector.tensor_reduce(ss[:], we[:], axis=mybir.AxisListType.X,
                                            op=OP.add)
                    rr = small.tile([P, 1], fp32, tag="rr")
                    nc.vector.reciprocal(rr[:], ss[:])
                    nc.vector.tensor_scalar(comb[:, ci, :], we[:], rr[:, 0:1],
                                            None, op0=OP.mult)

        # ------------------------------------------------------------------
        # Phase 2: dispatch construction per local expert
        # ------------------------------------------------------------------
        idx_i32 = [persist.tile([P, NMT], i32, tag=f"idx{le}", name=f"idx{le}")
                   for le in range(EPC)]
        w_sb = [persist.tile([P, NMT], fp32, tag=f"wsb{le}", name=f"wsb{le}")
                for le in range(EPC)]

        with tc.tile_pool(name="p2small", bufs=2, space="PSUM") as p2s:
            for le in range(EPC):
                me = sb.tile([P, NT], fp32, tag="me")
                nc.vector.tensor_scalar(me[:], comb[:, :, le], 0.0, None, op0=OP.is_gt)
                pp = p2s.tile([P, NT], fp32, tag="p2")
                nc.tensor.matmul(pp[:], lhsT=lt_sb[:], rhs=me[:], start=True, stop=False)
                pcs = p2s.tile([1, NT], fp32, tag="p2", name="pcs")
                nc.tensor.matmul(pcs[:], lhsT=ones_colp[:], rhs=me[:], start=True, stop=True)
                colsum = small.tile([1, NT], fp32, tag="colsum")
                nc.vector.tensor_copy(colsum[:], pcs[:])
                sc_a = small.tile([1, NT], fp32, tag="sc_a")
                sc_b = small.tile([1, NT], fp32, tag="sc_b")
                nc.vector.tensor_copy(sc_a[:], colsum[:])
                cur, nxt = sc_a, sc_b
                for sh in (1, 2, 4, 8):
                    nc.vector.tensor_copy(nxt[:, :sh], cur[:, :sh])
                    nc.vector.tensor_add(nxt[:, sh:], cur[:, sh:], cur[:, :NT - sh])
                    cur, nxt = nxt, cur
                cc = small.tile([1, NT], fp32, tag="cc")
                nc.vector.memset(cc[:, 0:1], 0.0)
                nc.vector.tensor_copy(cc[:, 1:], cur[:, :NT - 1])
                nc.tensor.matmul(pp[:], lhsT=ones_col[:], rhs=cc[:],
                                 start=False, stop=True)
                pm = sb.tile([P, NT], fp32, tag="pm")
                nc.vector.tensor_scalar(pm[:], pp[:], 1.0, None, op0=OP.add)
                nc.vector.tensor_mul(pm[:], pm[:], me[:])
                nc.vector.tensor_scalar(pm[:], pm[:], 1.0, None, op0=OP.subtract)

                rhs_all = sb.tile([P, NT, 4], bf16, tag="rhs_all")
                nc.vector.tensor_copy(rhs_all[:, :, 0], ktid_bf[:])
                nc.vector.tensor_copy(rhs_all[:, :, 1], pid_bf[:])
                nc.vector.tensor_copy(rhs_all[:, :, 2], comb[:, :, le])
                wbk = sb.tile([P, NT], fp32, tag="wbk")
                nc.vector.tensor_copy(wbk[:], rhs_all[:, :, 2])
                nc.vector.tensor_sub(wbk[:], comb[:, :, le], wbk[:])
                nc.vector.tensor_copy(rhs_all[:, :, 3], wbk[:])

                piw = p2s.tile([4, CAP], fp32, tag="p2", name="piw")
                for kt in range(NT):
                    sel = sb.tile([P, CAP], bf16, tag="sel")
                    nc.vector.tensor_scalar(sel[:], iota512f[:], pm[:, kt:kt + 1],
                                            None, op0=OP.is_equal)
                    nc.tensor.matmul(piw[:], lhsT=rhs_all[:, kt, :], rhs=sel[:],
                                     start=(kt == 0), stop=(kt == NT - 1))
                iw_sb = small.tile([4, CAP], fp32, tag="iw_sb")
                nc.vector.tensor_copy(iw_sb[:], piw[:])
                for mt in range(NMT):
                    ptr2 = p2s.tile([P, 4], fp32, tag="p2", name="ptr2")
                    nc.tensor.transpose(ptr2[:], iw_sb[:, mt * P:(mt + 1) * P],
                                        ident[:4, :4])
                    ptr2s = small.tile([P, 4], fp32, tag="ptr2s")
                    nc.vector.tensor_copy(ptr2s[:], ptr2[:])
                    idxf = small.tile([P, 1], fp32, tag="idxf")
                    nc.vector.scalar_tensor_tensor(idxf[:], in0=ptr2s[:, 0:1],
                                                   scalar=float(P), in1=ptr2s[:, 1:2],
                                                   op0=OP.mult, op1=OP.add)
                    nc.vector.tensor_copy(idx_i32[le][:, mt:mt + 1], idxf[:])
                    nc.vector.tensor_add(w_sb[le][:, mt:mt + 1], ptr2s[:, 2:3],
                                         ptr2s[:, 3:4])

        # ------------------------------------------------------------------
        # Phase 3a: shared expert (emitted early to keep PE warm while the
        # dispatch phase below runs on DVE/gpsimd)
        # ------------------------------------------------------------------
        pA = ctx.enter_context(tc.tile_pool(name="pA", bufs=3, space="PSUM"))
        pB = ctx.enter_context(tc.tile_pool(name="pB", bufs=3, space="PSUM"))

        ast = persist.tile([P, FSS // P, T], bf16)  # shared GEGLU output ^T
        for q in range(4):                          # token quarters of 512
            qs = slice(q * CAP, (q + 1) * CAP)
            for i in range(FSS // P):               # fs slice k-tiles (2)
                pxs = pA.tile([P, CAP], fp32, tag="shp")
                pgs = pA.tile([P, CAP], fp32, tag="shp")
                for kt in range(DKT):
                    nc.tensor.matmul(pxs[:], lhsT=s1w_sb[:, kt, i * P:(i + 1) * P],
                                     rhs=xbt[:, q, kt, :],
                                     start=(kt == 0), stop=(kt == DKT - 1))
                for kt in range(DKT):
                    nc.tensor.matmul(pgs[:], lhsT=s1w_sb[:, kt, FSS + i * P:FSS + (i + 1) * P],
                                     rhs=xbt[:, q, kt, :],
                                     start=(kt == 0), stop=(kt == DKT - 1))
                gel = sb.tile([P, CAP], fp32, tag="gel")
                nc.scalar.activation(gel[:], pgs[:], AF.Gelu,
                                     bias=s1b_sb[:, 2 + i:3 + i])
                nc.vector.scalar_tensor_tensor(ast[:, i, qs], in0=pxs[:],
                                               scalar=s1b_sb[:, i:i + 1],
                                               in1=gel[:], op0=OP.add, op1=OP.mult)
        for mt in range(NT):
            ys = ysp.tile([P, D], fp32, tag="ys")
            for h in range(2):
                hs = slice(h * 512, (h + 1) * 512)
                pys = pB.tile([P, 512], fp32, tag="pB")
                for i in range(FSS // P):
                    nc.tensor.matmul(pys[:], lhsT=ast[:, i, mt * P:(mt + 1) * P],
                                     rhs=s2w_sb[:, i, hs],
                                     start=(i == 0), stop=(i == FSS // P - 1))
                nc.vector.tensor_add(ys[:, hs], pys[:], s2bc[:, hs])
            nc.sync.dma_start(out=out[mt * P:(mt + 1) * P, :], in_=ys[:])

        # ------------------------------------------------------------------
        # Phase 3b: routed experts
        # ------------------------------------------------------------------
        pT = ctx.enter_context(tc.tile_pool(name="pT", bufs=2, space="PSUM"))
        xgt_t = []
        for le in range(EPC):
            xgt = apool.tile([P, DKT, CAP], bf16, tag="xgt", name=f"xgt{le}")
            xgt_t.append(xgt)
            for mt in range(NMT):
                xg = xgp.tile([P, D], bf16, tag="xg")
                nc.gpsimd.indirect_dma_start(
                    out=xg[:], out_offset=None, in_=xbf[:],
                    in_offset=bass.IndirectOffsetOnAxis(ap=idx_i32[le][:, mt:mt + 1], axis=0))
                for kt in range(DKT):
                    ptb = pT.tile([P, P], bf16, tag="ptb")
                    nc.tensor.transpose(ptb[:], xg[:, kt * P:(kt + 1) * P], ident_bf[:])
                    nc.vector.tensor_copy(xgt[:, kt, mt * P:(mt + 1) * P], ptb[:])
        for le in range(EPC):
            xgt = xgt_t[le]
            at = apool.tile([P, FKT, CAP], bf16, tag="at")
            for mf in range(FKT):
                w1blk = w1pool.tile([P, DKT, P], bf16, tag="w1")
                w1blk_g = w1pool.tile([P, DKT, P], bf16, tag="w1")
                nc.sync.dma_start(out=w1blk[:], in_=io["w1t"][le, mf])
                nc.sync.dma_start(out=w1blk_g[:], in_=io["w1t"][le, mf + FKT])
                pxh = pA.tile([P, CAP], fp32, tag="shp")
                pgg = pA.tile([P, CAP], fp32, tag="shp")
                for kt in range(DKT):
                    nc.tensor.matmul(pxh[:], lhsT=w1blk[:, kt, :], rhs=xgt[:, kt, :],
                                     start=(kt == 0), stop=(kt == DKT - 1))
                for kt in range(DKT):
                    nc.tensor.matmul(pgg[:], lhsT=w1blk_g[:, kt, :], rhs=xgt[:, kt, :],
                                     start=(kt == 0), stop=(kt == DKT - 1))
                gel = sb.tile([P, CAP], fp32, tag="gel")
                nc.scalar.activation(gel[:], pgg[:], AF.Gelu,
                                     bias=b1_sb[:, le, mf + FKT:mf + FKT + 1])
                nc.vector.scalar_tensor_tensor(at[:, mf, :], in0=pxh[:],
                                               scalar=b1_sb[:, le, mf:mf + 1],
                                               in1=gel[:], op0=OP.add, op1=OP.mult)
            for mt in range(NMT):
                yc = ycpool.tile([P, D], fp32, tag="yc")
                for h in range(2):
                    hs = slice(h * 512, (h + 1) * 512)
                    py = pB.tile([P, 512], fp32, tag="pB")
                    for kt in range(FKT):
                        nc.tensor.matmul(py[:], lhsT=at[:, kt, mt * P:(mt + 1) * P],
                                         rhs=w2_sb[le][:, kt, hs],
                                         start=(kt == 0), stop=(kt == FKT - 1))
                    nc.vector.tensor_add(yc[:, hs], py[:], b2bc[:, le, hs])
                    nc.vector.tensor_scalar(yc[:, hs], yc[:, hs], w_sb[le][:, mt:mt + 1],
                                            None, op0=OP.mult)
                nc.gpsimd.indirect_dma_start(
                    out=out[:], out_offset=bass.IndirectOffsetOnAxis(
                        ap=idx_i32[le][:, mt:mt + 1], axis=0),
                    in_=yc[:], in_offset=None,
                    compute_op=mybir.AluOpType.add)


# ----------------------------------------------------------------------------
# host-side input prep / sharding
# ----------------------------------------------------------------------------

def make_in_maps(inputs):
    bf = ml_dtypes.bfloat16
    x = np.ascontiguousarray(np.asarray(inputs["x"], np.float32).reshape(T, D))
    gate_w = np.asarray(inputs["gate_w"], np.float32)
    fc1_w = np.asarray(inputs["fc1_w"], np.float32)
    fc1_b = np.asarray(inputs["fc1_b"], np.float32)
    geglu = np.asarray(inputs["geglu_mult"], np.float32)
    fc2_w = np.asarray(inputs["fc2_w"], np.float32)
    fc2_b = np.asarray(inputs["fc2_b"], np.float32)
    s1w = np.asarray(inputs["s_fc1_w"], np.float32)
    s1b = np.asarray(inputs["s_fc1_b"], np.float32)
    sgeglu = np.asarray(inputs["s_geglu_mult"], np.float32)
    s2w = np.asarray(inputs["s_fc2_w"], np.float32)
    s2b = np.asarray(inputs["s_fc2_b"], np.float32)

    xbf = x.astype(bf)
    xer = (x - xbf.astype(np.float32)).astype(bf)
    ltm = np.triu(np.ones((P, P), np.float32), k=1)  # lt[r', r] = 1 iff r' < r

    in_maps = []
    for c in range(NC):
        local = [2 * c, 2 * c + 1] if c < NC - 1 else [14, -1]
        rest = [e for e in range(E) if e not in local]
        perm = (local + rest + [-1] * 16)[:16]

        gw = np.zeros((D, 16), np.float32)
        gb = np.zeros((P, 16), np.float32)
        for j, e in enumerate(perm):
            if e >= 0:
                gw[:, j] = gate_w[e]
            else:
                gb[:, j] = NEG
        gwb = gw.astype(bf)
        gwe = (gw - gwb.astype(np.float32)).astype(bf)

        w1t = np.zeros((EPC, NFT, P, DKT, P), bf)
        b1 = np.zeros((P, EPC, NFT), np.float32)
        w2t = np.zeros((EPC, P, FKT, D), bf)
        b2 = np.zeros((1, EPC, D), np.float32)
        for le in range(EPC):
            e = local[le]
            if e < 0:
                continue
            wt = fc1_w[e].T.astype(bf)          # [D, 2F]
            # w1t[le, mf, p, kt, fi] = wt[kt*128+p, mf*128+fi]
            w1t[le] = wt.reshape(DKT, P, NFT, P).transpose(2, 1, 0, 3)
            b1[:, le, :] = fc1_b[e].reshape(NFT, P).T
            w2 = (fc2_w[e] * geglu[e][None, :]).T.astype(bf)   # [F, D]
            w2t[le] = w2.reshape(FKT, P, D).transpose(1, 0, 2)
            b2[0, le, :] = fc2_b[e]

        fs0 = c * FSS
        s1 = np.concatenate([s1w[fs0:fs0 + FSS], s1w[FS + fs0:FS + fs0 + FSS]], 0)
        s1t = s1.T.astype(bf)                   # [D, 2*FSS]
        s1wt = s1t.reshape(DKT, P, 2 * FSS).transpose(1, 0, 2)
        s1bv = np.concatenate([s1b[fs0:fs0 + FSS], s1b[FS + fs0:FS + fs0 + FSS]])
        s1b_t = s1bv.reshape(4, P).T            # [128, 4]
        s2 = (s2w[:, fs0:fs0 + FSS] * sgeglu[None, fs0:fs0 + FSS]).T.astype(bf)
        s2wt = s2.reshape(FSS // P, P, D).transpose(1, 0, 2)
        s2bv = (s2b / NC).reshape(1, D).astype(np.float32)

        in_maps.append({
            "xbf": xbf, "xer": xer,
            "gwb": np.ascontiguousarray(gwb), "gwe": np.ascontiguousarray(gwe),
            "gbias": np.ascontiguousarray(gb), "ltm": ltm,
            "w1t": np.ascontiguousarray(w1t), "b1": np.ascontiguousarray(b1),
            "w2t": np.ascontiguousarray(w2t), "b2": np.ascontiguousarray(b2),
            "s1wt": np.ascontiguousarray(s1wt), "s1b": np.ascontiguousarray(s1b_t),
            "s2wt": np.ascontiguousarray(s2wt), "s2b": np.ascontiguousarray(s2bv),
        })
    return in_maps


def kernel(**inputs):
    if "nc" not in _prog_cache:
        _prog_cache["nc"] = build_program()
    nc = _prog_cache["nc"]
    in_maps = make_in_maps(inputs)
    from concourse.bass_utils import run_bass_kernel_spmd
    res = run_bass_kernel_spmd(nc, in_maps, core_ids=list(range(NC)))
    acc = np.zeros((T, D), np.float64)
    for r in res.results:
        acc += np.asarray(r["out"], np.float64)
    return acc.astype(np.float32).reshape(S, B, D)



# revision 22
# speedup vs baseline: 1.3244x; 1.3244x over previous
"""MoE (15 routed experts top-3 + shared GEGLU FFN) on 8 trn2 NeuronCores.

Strategy (expert-parallel + shared-expert tensor-parallel), v2:
  - Each core owns 2 routed experts (core 7: 1 real + 1 zero dummy) and a
    256-wide slice of the shared expert's FS=2048 hidden dim.
  - x is shipped both natural (gather source) and pre-transposed in bf16 +
    bf16-error form (gate compensation) -- no on-device DMA transposes.
  - PE warm-up uses real matmuls (transposes don't count for the HAM clock
    gate); gate chunks are interleaved with shared-fc1 chunks and dispatch
    small-ops are padded with shared-fc2 tiles so the PE never idles.
  - Token dispatch is built on-device with matmuls: top-3 via max8,
    per-expert cumsum via a triangular matmul, then a selection-matrix
    matmul extracts (token-id, weight) per capacity slot.
  - Routed expert rows are written densely (weighted, bf16) together with
    their token ids; the host does the final scatter-add combine across
    cores, which removes the serialized read-modify-write tail.
"""

import sys
import numpy as np

for _p in ("/opt/trn_rl_repo",):
    if _p not in sys.path:
        sys.path.insert(0, _p)

import ml_dtypes

S, B, D = 1024, 2, 1024
T = S * B                  # 2048 tokens
E, TOPK = 15, 3
F, FS = 1024, 2048
NC = 8                     # cores
EPC = 2                    # expert slots per core
CAP = 512                  # per-expert token capacity (max actual count ~463)
FSS = FS // NC             # shared-expert hidden slice per core = 256
NEG = -1.0e9

P = 128
DKT = D // P               # 8 k-tiles over D
FKT = F // P               # 8 k-tiles over F
NT = T // P                # 16 token tiles
NMT = CAP // P             # 4 capacity (slot) tiles per expert
NFT = 2 * F // P           # 16 f-tiles of fc1 output
NCH = 4                    # 512-token chunks
WARM = 26                  # PE warm-up matmuls

_prog_cache = {}


# ----------------------------------------------------------------------------
# device program
# ----------------------------------------------------------------------------

def build_program():
    import concourse.bass as bass
    import concourse.mybir as mybir
    import concourse.tile as tile
    from concourse import bacc
    from concourse.masks import make_identity

    fp32 = mybir.dt.float32
    bf16 = mybir.dt.bfloat16
    i32 = mybir.dt.int32

    nc = bacc.Bacc()

    xn = nc.dram_tensor("xn", [T, D], bf16, kind="ExternalInput")
    xt = nc.dram_tensor("xt", [NCH, P, DKT, 512], bf16, kind="ExternalInput")
    xet = nc.dram_tensor("xet", [NCH, P, DKT, 512], bf16, kind="ExternalInput")
    gw2 = nc.dram_tensor("gw2", [P, DKT, 48], bf16, kind="ExternalInput")
    gbias = nc.dram_tensor("gbias", [P, 16], fp32, kind="ExternalInput")
    ltm = nc.dram_tensor("ltm", [P, P], fp32, kind="ExternalInput")
    w1t = nc.dram_tensor("w1t", [EPC, NFT, P, DKT, P], bf16, kind="ExternalInput")
    b1 = nc.dram_tensor("b1", [P, EPC, NFT], fp32, kind="ExternalInput")
    w2t = nc.dram_tensor("w2t", [EPC, P, FKT, D], bf16, kind="ExternalInput")
    b2 = nc.dram_tensor("b2", [1, EPC, D], fp32, kind="ExternalInput")
    s1wt = nc.dram_tensor("s1wt", [P, DKT, 2 * FSS], bf16, kind="ExternalInput")
    s1b = nc.dram_tensor("s1b", [P, 4], fp32, kind="ExternalInput")
    s2wt = nc.dram_tensor("s2wt", [P, FSS // P, D], bf16, kind="ExternalInput")
    s2b = nc.dram_tensor("s2b", [1, D], fp32, kind="ExternalInput")
    outs = nc.dram_tensor("outs", [T, D], bf16, kind="ExternalOutput")
    yr = nc.dram_tensor("yr", [EPC, CAP, D], bf16, kind="ExternalOutput")
    idxo = nc.dram_tensor("idxo", [EPC, P, NMT], i32, kind="ExternalOutput")

    with tile.TileContext(nc) as tc:
        emit(nc, tc, tile, mybir, bass, make_identity, fp32, bf16, i32,
             dict(xn=xn, xt=xt, xet=xet, gw2=gw2, gbias=gbias, ltm=ltm,
                  w1t=w1t, b1=b1, w2t=w2t, b2=b2, s1wt=s1wt, s1b=s1b,
                  s2wt=s2wt, s2b=s2b, outs=outs, yr=yr, idxo=idxo))
    if not nc.is_finalized():
        nc.finalize()
    return nc


def emit(nc, tc, tile, mybir, bass, make_identity, fp32, bf16, i32, io):
    from contextlib import ExitStack

    AF = mybir.ActivationFunctionType
    OP = mybir.AluOpType

    ctx = ExitStack()
    with ctx:
        consts = ctx.enter_context(tc.tile_pool(name="consts", bufs=1))
        wpool = ctx.enter_context(tc.tile_pool(name="weights", bufs=1))
        xbt_pool = ctx.enter_context(tc.tile_pool(name="xbt", bufs=1))
        w1pool = ctx.enter_context(tc.tile_pool(name="w1", bufs=3))
        sb = ctx.enter_context(tc.tile_pool(name="sb", bufs=2))
        ysp = ctx.enter_context(tc.tile_pool(name="ysp", bufs=2))
        small = ctx.enter_context(tc.tile_pool(name="small", bufs=4))
        persist = ctx.enter_context(tc.tile_pool(name="persist", bufs=1))
        apool = ctx.enter_context(tc.tile_pool(name="apool", bufs=2))
        ycpool = ctx.enter_context(tc.tile_pool(name="ycpool", bufs=2))
        bbc = ctx.enter_context(tc.tile_pool(name="bbc", bufs=1))
        xgp = ctx.enter_context(tc.tile_pool(name="xgp", bufs=3))

        # ---- constants ----
        ident = consts.tile([P, P], fp32)
        make_identity(nc, ident[:])
        ident_bf = consts.tile([P, P], bf16)
        make_identity(nc, ident_bf[:])
        ones_col = consts.tile([1, P], fp32)
        nc.vector.memset(ones_col[:], 1.0)
        ones_colp = consts.tile([P, 1], fp32)
        nc.vector.memset(ones_colp[:], 1.0)
        junk = consts.tile([P, 512], bf16)
        nc.vector.memset(junk[:], 0.0)

        # PE warm-up with REAL matmuls (transpose-mode doesn't count for the
        # HAM activity monitor): keeps the PE busy ~0-6us so the clock gate
        # is released (2.4GHz) by the time the first gate chunk lands.
        warm_pool = tc.tile_pool(name="warm", bufs=2, space="PSUM")
        warm = warm_pool.__enter__()
        for _ in range(WARM):
            wt = warm.tile([P, 512], fp32, tag="wt")
            nc.tensor.matmul(wt[:], lhsT=ident_bf[:], rhs=junk[:],
                             start=True, stop=True)
        warm_pool.__exit__(None, None, None)

        # ---- input DMAs (sync ring, FIFO order == arrival order) ----
        gw2_sb = consts.tile([P, DKT, 48], bf16)
        nc.sync.dma_start(out=gw2_sb[:], in_=io["gw2"][:])
        gbias_sb = consts.tile([P, 16], fp32)
        nc.sync.dma_start(out=gbias_sb[:], in_=io["gbias"][:])

        xbt = xbt_pool.tile([P, NCH, DKT, 512], bf16)   # x^T, persists
        bias_stage = tc.alloc_tile_pool(name="bias_stage", bufs=1)
        xet_pool = tc.alloc_tile_pool(name="xet_pool", bufs=2)
        xet_t = []
        s1w_sb = wpool.tile([P, DKT, 2 * FSS], bf16)
        s2w_sb = wpool.tile([P, FSS // P, D], bf16)
        w2_sb = [wpool.tile([P, FKT, D], bf16, tag=f"w2_{le}", name=f"w2_{le}")
                 for le in range(EPC)]

        def _ldchunk(q):
            nc.sync.dma_start(out=xbt[:, q], in_=io["xt"][q])
            xe = xet_pool.tile([P, DKT, 512], bf16, tag="xet", name=f"xet{q}")
            nc.sync.dma_start(out=xe[:], in_=io["xet"][q])
            xet_t.append(xe)

        _ldchunk(0)
        nc.sync.dma_start(out=s1w_sb[:], in_=io["s1wt"][:])
        _ldchunk(1)
        _ldchunk(2)
        _ldchunk(3)

        s1b_sb = consts.tile([P, 4], fp32)
        nc.sync.dma_start(out=s1b_sb[:], in_=io["s1b"][:])
        s2b_sb = bias_stage.tile([1, D], fp32)
        nc.sync.dma_start(out=s2b_sb[:], in_=io["s2b"][:])
        b2_sb = bias_stage.tile([1, EPC, D], fp32)
        nc.sync.dma_start(out=b2_sb[:], in_=io["b2"][:])
        lt_sb = consts.tile([P, P], fp32)
        nc.sync.dma_start(out=lt_sb[:], in_=io["ltm"][:])
        b1_sb = consts.tile([P, EPC, NFT], fp32)
        nc.sync.dma_start(out=b1_sb[:], in_=io["b1"][:])
        nc.sync.dma_start(out=s2w_sb[:], in_=io["s2wt"][:])
        for le in range(EPC):
            nc.sync.dma_start(out=w2_sb[le][:], in_=io["w2t"][le])

        iota512f = consts.tile([P, CAP], fp32)
        with tc.tile_pool(name="iota_tmp", bufs=1) as iota_tmp:
            iota512 = iota_tmp.tile([P, CAP], i32)
            nc.gpsimd.iota(iota512[:], pattern=[[1, CAP]], base=0, channel_multiplier=0)
            nc.gpsimd.tensor_copy(iota512f[:], iota512[:])
        ktid = consts.tile([P, NT], i32)
        nc.gpsimd.iota(ktid[:], pattern=[[1, NT]], base=0, channel_multiplier=0)
        ktid_bf = consts.tile([P, NT], bf16)
        nc.gpsimd.tensor_copy(ktid_bf[:], ktid[:])
        pid = consts.tile([P, NT], i32)
        nc.gpsimd.iota(pid[:], pattern=[[0, NT]], base=0, channel_multiplier=1)
        pid_bf = consts.tile([P, NT], bf16)
        nc.gpsimd.tensor_copy(pid_bf[:], pid[:])

        # persistent activations
        comb = persist.tile([P, NT, 16], fp32)      # renormalized top-3 weights
        ast = persist.tile([P, FSS // P, T], bf16)  # shared GEGLU output ^T

        # ------------------------------------------------------------------
        # Phase 1: gate chunks interleaved with shared-fc1 chunks.
        # PE order: [gate ch] [sfc1 ch] [gate transposes ch] so the DVE
        # softmax/top-3 chain for chunk ch runs under chunk ch+1's matmuls.
        # ------------------------------------------------------------------
        pA = ctx.enter_context(tc.tile_pool(name="pA", bufs=3, space="PSUM"))

        with tc.tile_pool(name="p1lt", bufs=2, space="PSUM") as p1lt, \
             tc.tile_pool(name="p1tr", bufs=2, space="PSUM") as p1tr:
            lgt_t = []
            for ch in range(NCH):
                cs = slice(ch * 512, (ch + 1) * 512)
                xe = xet_t[ch]
                # gate logits (compensated bf16): rows 0-15 xb*B + xe*B,
                # rows 32-47 xb*E
                plt2 = p1lt.tile([48, 512], fp32, tag="plt2")
                for kt in range(DKT):
                    nc.tensor.matmul(plt2[:], lhsT=gw2_sb[:, kt, :],
                                     rhs=xbt[:, ch, kt, :],
                                     start=(kt == 0), stop=False)
                for kt in range(DKT):
                    nc.tensor.matmul(plt2[:16, :], lhsT=gw2_sb[:, kt, 0:16],
                                     rhs=xe[:, kt, :],
                                     start=False, stop=(kt == DKT - 1),
                                     skip_group_check=True)
                lgt_e = sb.tile([16, 512], fp32, tag="gel", name=f"lgt_e{ch}")
                nc.scalar.copy(lgt_e[:], plt2[32:48, :])
                lgt = sb.tile([16, 512], fp32, tag="lgt", name=f"lgt{ch}")
                nc.vector.tensor_add(lgt[:], plt2[:16, :], lgt_e[:])
                lgt_t.append(lgt)

                # shared fc1 for this token chunk
                for i in range(FSS // P):
                    pxs = pA.tile([P, 512], fp32, tag="shp")
                    pgs = pA.tile([P, 512], fp32, tag="shp")
                    for kt in range(DKT):
                        nc.tensor.matmul(pxs[:], lhsT=s1w_sb[:, kt, i * P:(i + 1) * P],
                                         rhs=xbt[:, ch, kt, :],
                                         start=(kt == 0), stop=(kt == DKT - 1))
                    for kt in range(DKT):
                        nc.tensor.matmul(pgs[:], lhsT=s1w_sb[:, kt, FSS + i * P:FSS + (i + 1) * P],
                                         rhs=xbt[:, ch, kt, :],
                                         start=(kt == 0), stop=(kt == DKT - 1))
                    gel = sb.tile([P, 512], fp32, tag="gel")
                    nc.scalar.activation(gel[:], pgs[:], AF.Gelu,
                                         bias=s1b_sb[:, 2 + i:3 + i])
                    nc.vector.scalar_tensor_tensor(ast[:, i, cs], in0=pxs[:],
                                                   scalar=s1b_sb[:, i:i + 1],
                                                   in1=gel[:], op0=OP.add, op1=OP.mult)

                # gate logit transposes + softmax/top-3 chain (DVE+gpsimd)
                for q in range(4):
                    ci = ch * 4 + q
                    ptr = p1tr.tile([P, 16], fp32, tag="ptr")
                    nc.tensor.transpose(ptr[:], lgt[:, q * P:(q + 1) * P],
                                        ident[:16, :16])
                    lg = sb.tile([P, 16], fp32, tag="lg")
                    nc.vector.tensor_add(lg[:], ptr[:], gbias_sb[:])
                    mx8 = small.tile([P, 8], fp32, tag="mx8")
                    nc.vector.max(out=mx8[:], in_=lg[:])
                    dd = small.tile([P, 16], fp32, tag="dd")
                    nc.vector.tensor_scalar(dd[:], lg[:], mx8[:, 0:1], None,
                                            op0=OP.subtract)
                    ee = small.tile([P, 16], fp32, tag="ee")
                    nc.scalar.activation(ee[:], dd[:], AF.Exp)
                    mm = small.tile([P, 16], fp32, tag="mm")
                    nc.vector.tensor_scalar(mm[:], lg[:], mx8[:, 2:3], None,
                                            op0=OP.is_ge)
                    we = small.tile([P, 16], fp32, tag="we")
                    nc.vector.tensor_mul(we[:], ee[:], mm[:])
                    ss = small.tile([P, 1], fp32, tag="ss")
                    nc.vector.tensor_reduce(ss[:], we[:], axis=mybir.AxisListType.X,
                                            op=OP.add)
                    rr = small.tile([P, 1], fp32, tag="rr")
                    nc.vector.reciprocal(rr[:], ss[:])
                    nc.vector.tensor_scalar(comb[:, ci, :], we[:], rr[:, 0:1],
                                            None, op0=OP.mult)
        xet_pool.release()

        # ------------------------------------------------------------------
        # Phase 2: dispatch per local expert, with shared-fc2 tiles as PE
        # filler between the latency chains.
        # ------------------------------------------------------------------
        idx_i32 = [persist.tile([P, NMT], i32, tag=f"idx{le}", name=f"idx{le}")
                   for le in range(EPC)]
        w_sb = [persist.tile([P, NMT], fp32, tag=f"wsb{le}", name=f"wsb{le}")
                for le in range(EPC)]

        pB = ctx.enter_context(tc.tile_pool(name="pB", bufs=2, space="PSUM"))

        # broadcast bias rows (ones outer-product) for shared/routed fc2
        s2bc = bbc.tile([P, D], bf16)
        b2bc = bbc.tile([P, EPC, D], bf16)

        sfc2_iter = iter(range(NT))

        def emit_sfc2(n):
            # n shared-fc2 token tiles: PE filler that is always ready
            for _ in range(n):
                mt = next(sfc2_iter, None)
                if mt is None:
                    return
                ys = ysp.tile([P, D], bf16, tag="ys")
                for h in range(2):
                    hs = slice(h * 512, (h + 1) * 512)
                    pys = pB.tile([P, 512], fp32, tag="pB")
                    for i in range(FSS // P):
                        nc.tensor.matmul(pys[:], lhsT=ast[:, i, mt * P:(mt + 1) * P],
                                         rhs=s2w_sb[:, i, hs],
                                         start=(i == 0), stop=(i == FSS // P - 1))
                    nc.vector.tensor_add(ys[:, hs], pys[:], s2bc[:, hs])
                nc.scalar.dma_start(out=io["outs"][mt * P:(mt + 1) * P, :], in_=ys[:])

        with tc.tile_pool(name="p2small", bufs=2, space="PSUM") as p2s:
            # bias broadcasts (cheap PE work emitted first)
            for h in range(2):
                hs = slice(h * 512, (h + 1) * 512)
                pc = p2s.tile([P, 512], fp32, tag="p2", name="pc")
                nc.tensor.matmul(pc[:], lhsT=ones_col[:], rhs=s2b_sb[:, hs],
                                 start=True, stop=True)
                nc.vector.tensor_copy(s2bc[:, hs], pc[:])
                for le in range(EPC):
                    pc2 = p2s.tile([P, 512], fp32, tag="p2", name="pc2")
                    nc.tensor.matmul(pc2[:], lhsT=ones_col[:], rhs=b2_sb[:, le, hs],
                                     start=True, stop=True)
                    nc.vector.tensor_copy(b2bc[:, le, hs], pc2[:])
            bias_stage.release()
            emit_sfc2(2)

            for le in range(EPC):
                me = sb.tile([P, NT], fp32, tag="me")
                nc.vector.tensor_scalar(me[:], comb[:, :, le], 0.0, None, op0=OP.is_gt)
                pp = p2s.tile([P, NT], fp32, tag="p2")
                nc.tensor.matmul(pp[:], lhsT=lt_sb[:], rhs=me[:], start=True, stop=False)
                pcs = p2s.tile([1, NT], fp32, tag="p2", name="pcs")
                nc.tensor.matmul(pcs[:], lhsT=ones_colp[:], rhs=me[:], start=True, stop=True)
                emit_sfc2(2)
                colsum = small.tile([1, NT], fp32, tag="colsum")
                nc.vector.tensor_copy(colsum[:], pcs[:])
                sc_a = small.tile([1, NT], fp32, tag="sc_a")
                sc_b = small.tile([1, NT], fp32, tag="sc_b")
                nc.vector.tensor_copy(sc_a[:], colsum[:])
                cur, nxt = sc_a, sc_b
                for sh in (1, 2, 4, 8):
                    nc.vector.tensor_copy(nxt[:, :sh], cur[:, :sh])
                    nc.vector.tensor_add(nxt[:, sh:], cur[:, sh:], cur[:, :NT - sh])
                    cur, nxt = nxt, cur
                cc = small.tile([1, NT], fp32, tag="cc")
                nc.vector.memset(cc[:, 0:1], 0.0)
                nc.vector.tensor_copy(cc[:, 1:], cur[:, :NT - 1])
                nc.tensor.matmul(pp[:], lhsT=ones_col[:], rhs=cc[:],
                                 start=False, stop=True)
                pm = sb.tile([P, NT], fp32, tag="pm")
                nc.vector.tensor_scalar(pm[:], pp[:], 1.0, None, op0=OP.add)
                nc.vector.tensor_mul(pm[:], pm[:], me[:])
                nc.vector.tensor_scalar(pm[:], pm[:], 1.0, None, op0=OP.subtract)

                rhs_all = sb.tile([P, NT, 4], bf16, tag="rhs_all")
                nc.gpsimd.tensor_copy(rhs_all[:, :, 0], ktid_bf[:])
                nc.gpsimd.tensor_copy(rhs_all[:, :, 1], pid_bf[:])
                nc.gpsimd.tensor_copy(rhs_all[:, :, 2], comb[:, :, le])
                wbk = sb.tile([P, NT], fp32, tag="wbk")
                nc.gpsimd.tensor_copy(wbk[:], rhs_all[:, :, 2])
                nc.gpsimd.tensor_sub(wbk[:], comb[:, :, le], wbk[:])
                nc.gpsimd.tensor_copy(rhs_all[:, :, 3], wbk[:])
                emit_sfc2(2)

                piw = p2s.tile([4, CAP], fp32, tag="p2", name="piw")
                for kt in range(NT):
                    sel = sb.tile([P, CAP], bf16, tag="sel")
                    nc.vector.tensor_scalar(sel[:], iota512f[:], pm[:, kt:kt + 1],
                                            None, op0=OP.is_equal)
                    nc.tensor.matmul(piw[:], lhsT=rhs_all[:, kt, :], rhs=sel[:],
                                     start=(kt == 0), stop=(kt == NT - 1))
                    if kt == 7:
                        emit_sfc2(2)
                iw_sb = sb.tile([4, CAP], fp32, tag="iw_sb")
                nc.vector.tensor_copy(iw_sb[:], piw[:])
                for mt in range(NMT):
                    ptr2 = p2s.tile([P, 4], fp32, tag="p2", name="ptr2")
                    nc.tensor.transpose(ptr2[:], iw_sb[:, mt * P:(mt + 1) * P],
                                        ident[:4, :4])
                    ptr2s = small.tile([P, 4], fp32, tag="ptr2s")
                    nc.vector.tensor_copy(ptr2s[:], ptr2[:])
                    idxf = small.tile([P, 1], fp32, tag="idxf")
                    nc.vector.scalar_tensor_tensor(idxf[:], in0=ptr2s[:, 0:1],
                                                   scalar=float(P), in1=ptr2s[:, 1:2],
                                                   op0=OP.mult, op1=OP.add)
                    nc.gpsimd.tensor_copy(idx_i32[le][:, mt:mt + 1], idxf[:])
                    nc.gpsimd.tensor_add(w_sb[le][:, mt:mt + 1], ptr2s[:, 2:3],
                                         ptr2s[:, 3:4])
                nc.scalar.dma_start(out=io["idxo"][le], in_=idx_i32[le][:])
                emit_sfc2(3)

            emit_sfc2(NT)  # any remaining shared-fc2 tiles

        # ------------------------------------------------------------------
        # Phase 3: routed experts. Gather (gpsimd) -> PE transpose -> fc1 ->
        # GEGLU -> fc2 -> weighted bf16 rows to DRAM (no RMW scatter).
        # ------------------------------------------------------------------
        pT = ctx.enter_context(tc.tile_pool(name="pT", bufs=2, space="PSUM"))
        xgt_t = []

        def emit_gather(le):
            xgt = apool.tile([P, DKT, CAP], bf16, tag="xgt", name=f"xgt{le}")
            xgt_t.append(xgt)
            for mt in range(NMT):
                xg = xgp.tile([P, D], bf16, tag="xg")
                nc.gpsimd.indirect_dma_start(
                    out=xg[:], out_offset=None, in_=io["xn"][:],
                    in_offset=bass.IndirectOffsetOnAxis(ap=idx_i32[le][:, mt:mt + 1], axis=0))
                for kt in range(DKT):
                    ptb = pT.tile([P, P], bf16, tag="ptb")
                    nc.tensor.transpose(ptb[:], xg[:, kt * P:(kt + 1) * P], ident_bf[:])
                    nc.vector.tensor_copy(xgt[:, kt, mt * P:(mt + 1) * P], ptb[:])

        emit_gather(0)
        emit_gather(1)

        at_t = []
        for le in range(EPC):
            xgt = xgt_t[le]
            at = apool.tile([P, FKT, CAP], bf16, tag="at", name=f"at{le}")
            at_t.append(at)
            for mf in range(FKT):
                w1blk = w1pool.tile([P, DKT, P], bf16, tag="w1")
                w1blk_g = w1pool.tile([P, DKT, P], bf16, tag="w1")
                nc.sync.dma_start(out=w1blk[:], in_=io["w1t"][le, mf])
                nc.sync.dma_start(out=w1blk_g[:], in_=io["w1t"][le, mf + FKT])
                pxh = pA.tile([P, CAP], fp32, tag="shp")
                pgg = pA.tile([P, CAP], fp32, tag="shp")
                for kt in range(DKT):
                    nc.tensor.matmul(pxh[:], lhsT=w1blk[:, kt, :], rhs=xgt[:, kt, :],
                                     start=(kt == 0), stop=(kt == DKT - 1))
                for kt in range(DKT):
                    nc.tensor.matmul(pgg[:], lhsT=w1blk_g[:, kt, :], rhs=xgt[:, kt, :],
                                     start=(kt == 0), stop=(kt == DKT - 1))
                gel = sb.tile([P, CAP], fp32, tag="gel")
                nc.scalar.activation(gel[:], pgg[:], AF.Gelu,
                                     bias=b1_sb[:, le, mf + FKT:mf + FKT + 1])
                nc.vector.scalar_tensor_tensor(at[:, mf, :], in0=pxh[:],
                                               scalar=b1_sb[:, le, mf:mf + 1],
                                               in1=gel[:], op0=OP.add, op1=OP.mult)
            for mt in range(NMT):
                yc = ycpool.tile([P, D], bf16, tag="yc")
                yc_bf = ycpool.tile([P, D], bf16, tag="ycbf")
                for h in range(2):
                    hs = slice(h * 512, (h + 1) * 512)
                    py = pB.tile([P, 512], fp32, tag="pB")
                    for kt in range(FKT):
                        nc.tensor.matmul(py[:], lhsT=at[:, kt, mt * P:(mt + 1) * P],
                                         rhs=w2_sb[le][:, kt, hs],
                                         start=(kt == 0), stop=(kt == FKT - 1))
                    nc.vector.tensor_add(yc[:, hs], py[:], b2bc[:, le, hs])
                    nc.vector.tensor_scalar(yc_bf[:, hs], yc[:, hs],
                                            w_sb[le][:, mt:mt + 1],
                                            None, op0=OP.mult)
                nc.scalar.dma_start(out=io["yr"][le, mt * P:(mt + 1) * P, :],
                                    in_=yc_bf[:])


# ----------------------------------------------------------------------------
# host-side input prep / sharding
# ----------------------------------------------------------------------------

def make_in_maps(inputs):
    bf = ml_dtypes.bfloat16
    x = np.ascontiguousarray(np.asarray(inputs["x"], np.float32).reshape(T, D))
    gate_w = np.asarray(inputs["gate_w"], np.float32)
    fc1_w = np.asarray(inputs["fc1_w"], np.float32)
    fc1_b = np.asarray(inputs["fc1_b"], np.float32)
    geglu = np.asarray(inputs["geglu_mult"], np.float32)
    fc2_w = np.asarray(inputs["fc2_w"], np.float32)
    fc2_b = np.asarray(inputs["fc2_b"], np.float32)
    s1w = np.asarray(inputs["s_fc1_w"], np.float32)
    s1b = np.asarray(inputs["s_fc1_b"], np.float32)
    sgeglu = np.asarray(inputs["s_geglu_mult"], np.float32)
    s2w = np.asarray(inputs["s_fc2_w"], np.float32)
    s2b = np.asarray(inputs["s_fc2_b"], np.float32)

    xbf = x.astype(bf)
    xe32 = x - xbf.astype(np.float32)
    # x^T chunked: [NCH, P, DKT, 512];  xt4[ch,p,kt,j] = x[ch*512+j, kt*128+p]
    def tchunk(a):
        # a: [T, D] fp32 -> [NCH, P, DKT, 512] bf16
        return np.ascontiguousarray(
            a.reshape(NCH, 512, DKT, P).transpose(0, 3, 2, 1).astype(bf))
    xt4 = tchunk(x)
    xet4 = tchunk(xe32)
    ltm = np.triu(np.ones((P, P), np.float32), k=1)  # lt[r', r] = 1 iff r' < r

    in_maps = []
    for c in range(NC):
        local = [2 * c, 2 * c + 1] if c < NC - 1 else [14, -1]
        rest = [e for e in range(E) if e not in local]
        perm = (local + rest + [-1] * 16)[:16]

        gw = np.zeros((D, 16), np.float32)
        gb = np.zeros((P, 16), np.float32)
        for j, e in enumerate(perm):
            if e >= 0:
                gw[:, j] = gate_w[e]
            else:
                gb[:, j] = NEG
        gwb = gw.astype(bf)
        gwe = (gw - gwb.astype(np.float32)).astype(bf)
        gw2 = np.zeros((P, DKT, 48), bf)
        # gw2[p, kt, j] = gwb[kt*128+p, j] (cols 0-15) / gwe (cols 32-47)
        gw2[:, :, 0:16] = gwb.reshape(DKT, P, 16).transpose(1, 0, 2)
        gw2[:, :, 32:48] = gwe.reshape(DKT, P, 16).transpose(1, 0, 2)

        w1t = np.zeros((EPC, NFT, P, DKT, P), bf)
        b1 = np.zeros((P, EPC, NFT), np.float32)
        w2t = np.zeros((EPC, P, FKT, D), bf)
        b2 = np.zeros((1, EPC, D), np.float32)
        for le in range(EPC):
            e = local[le]
            if e < 0:
                continue
            wt = fc1_w[e].T.astype(bf)          # [D, 2F]
            # w1t[le, mf, p, kt, fi] = wt[kt*128+p, mf*128+fi]
            w1t[le] = wt.reshape(DKT, P, NFT, P).transpose(2, 1, 0, 3)
            b1[:, le, :] = fc1_b[e].reshape(NFT, P).T
            w2 = (fc2_w[e] * geglu[e][None, :]).T.astype(bf)   # [F, D]
            w2t[le] = w2.reshape(FKT, P, D).transpose(1, 0, 2)
            b2[0, le, :] = fc2_b[e]

        fs0 = c * FSS
        s1 = np.concatenate([s1w[fs0:fs0 + FSS], s1w[FS + fs0:FS + fs0 + FSS]], 0)
        s1t = s1.T.astype(bf)                   # [D, 2*FSS]
        s1wt = s1t.reshape(DKT, P, 2 * FSS).transpose(1, 0, 2)
        s1bv = np.concatenate([s1b[fs0:fs0 + FSS], s1b[FS + fs0:FS + fs0 + FSS]])
        s1b_t = s1bv.reshape(4, P).T            # [128, 4]
        s2 = (s2w[:, fs0:fs0 + FSS] * sgeglu[None, fs0:fs0 + FSS]).T.astype(bf)
        s2wt = s2.reshape(FSS // P, P, D).transpose(1, 0, 2)
        s2bv = (s2b / NC).reshape(1, D).astype(np.float32)

        in_maps.append({
            "xn": xbf, "xt": xt4, "xet": xet4,
            "gw2": np.ascontiguousarray(gw2),
            "gbias": np.ascontiguousarray(gb), "ltm": ltm,
            "w1t": np.ascontiguousarray(w1t), "b1": np.ascontiguousarray(b1),
            "w2t": np.ascontiguousarray(w2t), "b2": np.ascontiguousarray(b2),
            "s1wt": np.ascontiguousarray(s1wt), "s1b": np.ascontiguousarray(s1b_t),
            "s2wt": np.ascontiguousarray(s2wt), "s2b": np.ascontiguousarray(s2bv),
        })
    return in_maps


def kernel(**inputs):
    if "nc" not in _prog_cache:
        _prog_cache["nc"] = build_program()
    nc = _prog_cache["nc"]
    in_maps = make_in_maps(inputs)
    from concourse.bass_utils import run_bass_kernel_spmd
    res = run_bass_kernel_spmd(nc, in_maps, core_ids=list(range(NC)))
    acc = np.zeros((T, D), np.float64)
    idx_all = []
    row_all = []
    for r in res.results:
        acc += np.asarray(r["outs"], np.float64)
        yr = np.asarray(r["yr"], np.float32)        # [EPC, CAP, D]
        idxo = np.asarray(r["idxo"], np.int64)      # [EPC, P, NMT]
        for le in range(EPC):
            # slot (mt*P + p) -> token idxo[le, p, mt]
            idx_all.append(idxo[le].T.reshape(-1))  # [NMT*P] slot-ordered
            row_all.append(yr[le])
    idx_all = np.concatenate(idx_all)
    row_all = np.concatenate(row_all, axis=0).astype(np.float64)
    np.add.at(acc, idx_all, row_all)
    return acc.astype(np.float32).reshape(S, B, D)
